# revision 2
# baseline (speedup 1.0000x reference)
"""Trainium2 Bass kernel for nn_Head: single-head self-attention where
q = k = v = x @ Wq + bq and softmax is over the *query* axis (dim 1).

Math (per batch b):
    Q = X @ Wq + bq                       [T, D]
    S = Q @ Q^T / sqrt(D)                 [T, T]   (symmetric!)
    W = softmax(S, axis=0)                (normalize over rows i per column j)
    A[i, d] = sum_j W[i, j] Q[j, d]

Because S is symmetric, the column softmax stats (sums over i for column j)
are row sums of row j.  With E = exp(S/8) and l_j = sum_i E[j, i]:
    A^T = sum_j (Q_j / l_j)^T @ E_j       (E_j = 128-row tiles of E)
a clean matmul accumulation with no online-softmax rescaling.  Logits are
small (|S|/8 <~ 3), so no max subtraction is needed.

This execution environment charges a large fixed cost PER INSTRUCTION
(dispatch-dominated, nearly independent of operand size), so the kernel
minimizes instruction count rather than classical FLOP/byte rooflines:

  - host pre-casts x to bf16 and pre-transposes it to [E, T] (layout prep
    only), so the device needs no transposes/casts of X: ONE plain DMA
    loads both batches
  - projection: 32 matmuls (Wq k-slice loaded once per ko for both
    batches) into 4-bank PSUM tiles + one fused bias-add per batch (DVE,
    writes bf16)
  - Q in natural [t, d] layout via ONE SBUF->SBUF DMA-xbar-transpose
  - S tiles: 4 matmuls per 128-row tile into a 4-bank PSUM tile, then one
    ACT exp instruction [128, 2048] with the row-sum fused via accum_out
  - A^T accumulated with 64 matmuls per batch (PE-array weights reused
    across the 4 column chunks of each j; redundant LDWEIGHTS removed by
    a post-pass)
  - A^T [64, 2048] bf16 is DMA'd out as-is; the host transposes + upcasts

Sharding: data-parallel over batch, 2 batches per core, 8 cores, no
collectives.  Full inputs in, full output out.
"""

import numpy as np
import ml_dtypes

import concourse.bass as bass
import concourse.mybir as mybir
import concourse.tile as tile
from concourse.bass import ts
from concourse.bass_utils import run_bass_kernel_spmd

B, T, E, D = 16, 2048, 512, 64
NCORES = 8
BPC = B // NCORES  # batches per core
P = 128
NJ = T // P     # 16 row-tiles
NKO = E // P    # 4 contraction tiles for the projection
NCH = T // 512  # 4 PSUM-bank column chunks

f32 = mybir.dt.float32
bf16 = mybir.dt.bfloat16
EXP = mybir.ActivationFunctionType.Exp
BF = ml_dtypes.bfloat16


def split_multi_waits(nc: bass.Bass) -> int:
    """This walrus build rejects >1 sync-wait per instruction: split any
    extra waits into preceding single-wait EventSemaphore instructions on
    the same (in-order) engine queue."""
    import bass_rust

    n_split = 0
    for f in nc.m.functions:
        for blk in f.blocks:
            insts = blk.instructions
            if not any(
                i.sync_info is not None and len(i.sync_info.on_wait) > 1
                for i in insts
            ):
                continue
            new_list = []
            for ins in insts:
                si = ins.sync_info
                if si is not None and len(si.on_wait) > 1:
                    waits = list(si.on_wait)
                    for k, w in enumerate(waits[:-1]):
                        e = mybir.InstEventSemaphore(
                            name=f"wsplit_{ins.name}_{k}", ins=[], outs=[]
                        )
                        e.engine = ins.engine
                        e.sync_info = bass_rust.SyncInfo(on_wait=[w], on_update=[])
                        new_list.append(e)
                        n_split += 1
                    si.on_wait = waits[-1:]
                new_list.append(ins)
            blk.instructions = new_list
    return n_split


def dedup_ldweights(nc: bass.Bass) -> int:
    """Remove InstLdweights identical to the immediately-preceding one on
    the PE queue (with only plain matmuls/sems/drains in between).  The PE
    array retains its loaded weights, so the reload is redundant.  Only
    sync-free duplicates are removed so all semaphore waits/updates are
    preserved."""
    n_removed = 0
    for f in nc.m.functions:
        for blk in f.blocks:
            new_list = []
            last_sig = None
            for ins in blk.instructions:
                if ins.engine != mybir.EngineType.PE:
                    new_list.append(ins)
                    continue
                tn = type(ins).__name__
                if tn == "InstLdweights":
                    a = ins.ins[0]
                    sig = (str(a.ap), str(a.offset), str(a.dtype),
                           str(ins.perf_mode), str(ins.is_transpose),
                           str(ins.tile_position))
                    si = ins.sync_info
                    clean = si is None or (not si.on_wait and not si.on_update)
                    if sig == last_sig and clean:
                        n_removed += 1
                        continue
                    last_sig = sig
                    new_list.append(ins)
                elif tn == "InstMatmult" and not ins.is_transpose:
                    new_list.append(ins)
                elif tn in ("InstEventSemaphore", "InstDrain"):
                    new_list.append(ins)
                else:
                    last_sig = None
                    new_list.append(ins)
            blk.instructions = new_list
    return n_removed


def build_module(reps: int = 1) -> bass.Bass:
    nc = bass.Bass("TRN2", target_bir_lowering=False, debug=False,
                   num_devices=NCORES)
    xt = nc.declare_dram_parameter("xt", [BPC, E, T], bf16, isOutput=False).ap()
    wq = nc.declare_dram_parameter("wq", [P, NKO, D], bf16, isOutput=False).ap()
    bq = nc.declare_dram_parameter("bq", [D, 1], f32, isOutput=False).ap()
    at_out = nc.declare_dram_parameter("at", [BPC, D, T], bf16, isOutput=True).ap()

    with tile.TileContext(nc) as tc:
        with (
            tc.tile_pool(name="consts", bufs=1) as consts,
            tc.tile_pool(name="xt_p", bufs=2) as xt_p,
            tc.tile_pool(name="qtb_p", bufs=2) as qtb_p,
            tc.tile_pool(name="qn_p", bufs=2) as qn_p,
            tc.tile_pool(name="l_p", bufs=4) as l_p,
            tc.tile_pool(name="e_p", bufs=NJ + 2) as e_p,
            tc.tile_pool(name="ab_p", bufs=2) as ab_p,
            tc.tile_pool(name="ps_big", bufs=2, space="PSUM") as ps_big,
        ):
            wqs = consts.tile([P, NKO, D], bf16)
            nc.sync.dma_start(out=wqs[:], in_=wq)
            bq_sb = consts.tile([D, 1], f32)
            nc.sync.dma_start(out=bq_sb[:], in_=bq)

            for rep in range(reps):
                u = f"{rep}"
                # ---- input: X^T bf16 for both batches in one DMA
                xts = xt_p.tile([P, BPC, NKO, T], bf16, tag="xts", name=f"xts{u}")
                nc.sync.dma_start(
                    out=xts[:], in_=xt.rearrange("b (ko p) t -> p b ko t", p=P))

                # ---- projection: QT[d, t] = sum_e Wq[e, d] X^T[e, t] (+bq)
                # ko outermost so each Wq k-slice is loaded into the PE
                # array once for both batches (LDWEIGHTS dedup)
                qtb = qtb_p.tile([D, BPC, T], bf16, tag="qtb", name=f"qtb{u}")
                qt_pss = [ps_big.tile([P, T], f32, tag="big", name=f"qtps{u}_{b}")
                          for b in range(BPC)]
                for ko in range(NKO):
                    for b in range(BPC):
                        for c in range(NCH):
                            nc.tensor.matmul(
                                qt_pss[b][0:D, ts(c, 512)],
                                lhsT=wqs[:, ko, :],
                                rhs=xts[:, b, ko, ts(c, 512)],
                                start=(ko == 0),
                                stop=(ko == NKO - 1),
                                skip_group_check=True,
                            )
                for b in range(BPC):
                    nc.vector.tensor_scalar_add(
                        qtb[:, b, :], qt_pss[b][0:D, :], bq_sb[:])

                # ---- Q natural layout via one DMA xbar transpose (both
                # batches): transposed row r = (b*NJ + j)*128 + p lands at
                # qn[p, b, j, :]
                qn = qn_p.tile([P, BPC, NJ, D], bf16, tag="qn", name=f"qn{u}")
                nc.sync.dma_start(
                    out=qn[:], in_=qtb[:].rearrange("d b t -> d (b t)"),
                    transpose=True)

                # ---- per batch: S row-tiles + exp, then Qs, then A^T
                ab = ab_p.tile([D, BPC, T], bf16, tag="ab", name=f"ab{u}")
                for b in range(BPC):
                    l_all = l_p.tile([P, NJ], f32, tag="l", name=f"l{u}_{b}")
                    e_tiles = []
                    for j in range(NJ):
                        s_ps = ps_big.tile([P, T], f32, tag="big",
                                           name=f"sps{u}_{b}_{j}")
                        lhs = qtb[:, b, ts(j, P)]
                        for c in range(NCH):
                            nc.tensor.matmul(
                                s_ps[:, ts(c, 512)], lhsT=lhs,
                                rhs=qtb[:, b, ts(c, 512)], start=True, stop=True)
                        et = e_p.tile([P, T], bf16, tag="E", name=f"e{u}_{b}_{j}")
                        e_tiles.append(et)
                        nc.scalar.activation(
                            et[:], s_ps[:], EXP, bias=0.0, scale=0.125,
                            accum_out=l_all[:, j:j + 1])

                    lr = l_p.tile([P, NJ], f32, tag="lr", name=f"lr{u}_{b}")
                    nc.vector.reciprocal(lr[:], l_all[:])
                    qs = qn_p.tile([P, NJ, D], bf16, tag="qs", name=f"qs{u}_{b}")
                    nc.vector.tensor_mul(
                        qs[:], qn[:, b, :, :],
                        lr[:].unsqueeze(2).broadcast_to([P, NJ, D]))

                    at_ps = ps_big.tile([P, T], f32, tag="big",
                                        name=f"atps{u}_{b}")
                    for j in range(NJ):
                        for c in range(NCH):
                            nc.tensor.matmul(
                                at_ps[0:D, ts(c, 512)],
                                lhsT=qs[:, j, :],
                                rhs=e_tiles[j][:, ts(c, 512)],
                                start=(j == 0),
                                stop=(j == NJ - 1),
                                skip_group_check=True,
                            )
                    nc.vector.tensor_copy(ab[:, b, :], at_ps[0:D, :])
                nc.sync.dma_start(
                    out=at_out.rearrange("b d t -> d b t"), in_=ab[:])

    dedup_ldweights(nc)
    split_multi_waits(nc)
    return nc


def make_in_maps(x: np.ndarray, Wq: np.ndarray, bq: np.ndarray):
    """Host-side layout prep (pure casts/reshapes, no math)."""
    xbf = np.ascontiguousarray(x.transpose(0, 2, 1)).astype(BF)  # [B, E, T]
    wqh = np.ascontiguousarray(
        Wq.reshape(NKO, P, D).transpose(1, 0, 2)).astype(BF)     # [128, 4, 64]
    bqh = np.ascontiguousarray(bq.reshape(D, 1)).astype(np.float32)
    return [
        {"xt": np.ascontiguousarray(xbf[i * BPC:(i + 1) * BPC]),
         "wq": wqh, "bq": bqh}
        for i in range(NCORES)
    ]


def _gather(res) -> np.ndarray:
    at = np.concatenate([res.results[i]["at"] for i in range(NCORES)], axis=0)
    return np.ascontiguousarray(at.transpose(0, 2, 1)).astype(np.float32)


def kernel(x: np.ndarray, Wq: np.ndarray, bq: np.ndarray) -> np.ndarray:
    assert x.shape == (B, T, E) and Wq.shape == (E, D) and bq.shape == (D,)
    in_maps = make_in_maps(np.asarray(x), np.asarray(Wq), np.asarray(bq))
    last_exc = None
    for _attempt in range(3):
        try:
            nc = build_module()
            res = run_bass_kernel_spmd(nc, in_maps, core_ids=list(range(NCORES)))
            return _gather(res)
        except Exception as e:  # transient device wedge: rebuild + retry
            last_exc = e
            import time as _time
            _time.sleep(5.0)
    raise last_exc
